# revision 3
# baseline (speedup 1.0000x reference)
"""GRPO loss kernel for Trainium2 (8 NeuronCores, data-parallel over B*L rows).

Heavy part: per-row logsumexp over the vocab dim of logits (2, 1025, 50257) f32.
Rows (B*L = 2048) are sharded 256/core; each core streams its (256, 50257) slab
through SBUF and computes per-row sum(exp(x)) with fused ACT exp+accumulate.
Host finishes with log(), the token-logit gather, and the tiny (B, L) epilogue.
"""

import sys
import types

import numpy as np


def _ensure_axon_hooks():
    """bass_utils imports antenv.axon_hooks when tracing is requested (e.g.
    BASS_TRACE=1); this image's antenv lacks that module. Install the same
    hook trn_boot would, so a traced run profiles instead of crashing."""
    try:
        import antenv.axon_hooks  # noqa: F401
        return
    except ImportError:
        pass
    hook = [None]
    mod = types.ModuleType("antenv.axon_hooks")
    mod.set_axon_ntff_profile_hook = lambda h: hook.__setitem__(0, h)
    mod.get_axon_ntff_profile_hook = lambda: hook[0]
    try:
        import antenv
        sys.modules["antenv.axon_hooks"] = mod
        antenv.axon_hooks = mod
        import trn_agent_boot.trn_boot as tb
        mod.set_axon_ntff_profile_hook(
            tb._ntff_profile_via_ctypes("/opt/axon/libaxon_pjrt.so"))
    except Exception:
        pass


_ensure_axon_hooks()

import concourse.bacc as bacc  # noqa: E402
import concourse.tile as tile  # noqa: E402
from concourse import bass_utils  # noqa: E402
from concourse import mybir  # noqa: E402
from concourse.bass_utils import run_bass_kernel_spmd  # noqa: E402

# upload_artifacts copies the NEFF dir to a fish bucket; in sandboxes without
# bucket access that throws and kills a traced run. Fall back to the local dir.
_orig_upload = bass_utils.upload_artifacts


def _safe_upload(tmpdir):
    try:
        return _orig_upload(tmpdir)
    except Exception:
        return tmpdir


bass_utils.upload_artifacts = _safe_upload

B = 2
L = 1024
V = 50257
TEMPERATURE = 1.0
BETA = 0.04
EPS_LOW = 0.2
EPS_HIGH = 0.2

N_CORES = 8
ROWS_PER_CORE = (B * L) // N_CORES  # 256
P = 128                             # SBUF partitions
PT_TILES = ROWS_PER_CORE // P       # 2
# 32KB per-partition descriptors (under the 64KB SDMA limit): the DMA queue
# processing engine (#79) also carries the descriptor ring, so fewer/bigger
# descriptors per byte keeps it from straggling ~20us behind the other 15.
FREE = 8192                         # free-dim tile -> 4 MiB DMAs
N_FT = (V + FREE - 1) // FREE       # 7 (6 full + 1105 tail)
BUFS = 6                            # 6 x 32KB = 192KB/partition

_cache = {}


def _build_nc():
    # Bacc (not raw Bass): its compile() pass splits multi-sem waits into
    # EventSemaphore instructions — TRN2 allows only 1 wait per instruction.
    nc = bacc.Bacc("TRN2", target_bir_lowering=False)
    x = nc.dram_tensor("x", [ROWS_PER_CORE, V], mybir.dt.float32,
                       kind="ExternalInput")
    out = nc.dram_tensor("partials", [ROWS_PER_CORE, N_FT], mybir.dt.float32,
                         kind="ExternalOutput")

    with tile.TileContext(nc) as tc:
        with (
            tc.tile_pool(name="xtiles", bufs=BUFS) as xpool,
            tc.tile_pool(name="stats", bufs=2) as spool,
        ):
            for pt in range(PT_TILES):
                partials = spool.tile([P, N_FT], mybir.dt.float32)
                for ft in range(N_FT):
                    f0 = ft * FREE
                    w = min(V - f0, FREE)
                    xt = xpool.tile([P, FREE], mybir.dt.float32)
                    nc.sync.dma_start(
                        out=xt[:, :w],
                        in_=x[pt * P:(pt + 1) * P, f0:f0 + w],
                    )
                    nc.scalar.activation(
                        out=xt[:, :w],
                        in_=xt[:, :w],
                        func=mybir.ActivationFunctionType.Exp,
                        accum_out=partials[:, ft:ft + 1],
                    )
                # outputs ride the (otherwise idle) ACT HWDGE ring: never
                # queue behind in-flight loads on the sync ring, and skip the
                # ~1.7us gpsimd drain the SWDGE path puts on the tail
                nc.scalar.dma_start(out=out[pt * P:(pt + 1) * P], in_=partials)
    nc.finalize()
    return nc


def _get_nc():
    if "nc" not in _cache:
        _cache["nc"] = _build_nc()
    return _cache["nc"]


def _run_device(logits, trace=False):
    """Returns per-row sum(exp(logit)) of shape (B*L,), plus the raw result."""
    # Each core's 256 rows sit inside one batch row of logits, so the shard
    # logits[b, l0:l0+256, :] is a contiguous zero-copy view (no 412MB copy).
    cores_per_b = N_CORES // B
    in_maps = []
    for i in range(N_CORES):
        b, l0 = i // cores_per_b, (i % cores_per_b) * ROWS_PER_CORE
        shard = logits[b, l0:l0 + ROWS_PER_CORE, :].astype(np.float32,
                                                           copy=False)
        in_maps.append({"x": np.ascontiguousarray(shard)})
    res = run_bass_kernel_spmd(_get_nc(), in_maps,
                               core_ids=list(range(N_CORES)), trace=trace)
    part = np.stack([r["partials"] for r in res.results])   # (8, 256, N_FT)
    sumexp = part.astype(np.float64).sum(axis=-1).reshape(B * L)
    return sumexp, res


def kernel(logits, completion_ids, advantages, old_logp, ref_logp,
           completion_mask, _trace=False, _want_res=False):
    logits = np.asarray(logits)
    completion_ids = np.asarray(completion_ids)
    advantages = np.asarray(advantages)
    old_logp = np.asarray(old_logp)
    ref_logp = np.asarray(ref_logp)
    completion_mask = np.asarray(completion_mask)

    sumexp, res = _run_device(logits, trace=_trace)

    lse = np.log(sumexp).reshape(B, L).astype(np.float32)        # (B, L)
    tok_logit = np.take_along_axis(
        logits[:, :L, :], completion_ids[..., None].astype(np.int64), axis=2
    )[..., 0].astype(np.float32)
    if TEMPERATURE != 1.0:
        tok_logit = tok_logit / np.float32(TEMPERATURE)
    logp = tok_logit - lse                                       # (B, L)

    coef_1 = np.exp(logp - old_logp)
    adv = advantages[:, None].astype(np.float32)                 # (B, 1)
    coef_2 = np.clip(coef_1, 1.0 - EPS_LOW, 1.0 + EPS_HIGH)
    loss1 = coef_1 * adv
    loss2 = coef_2 * adv
    per_token_loss = -np.minimum(loss1, loss2)

    diff = ref_logp.astype(np.float32) - logp
    kl = np.exp(diff) - diff - 1.0
    per_token_loss = per_token_loss + np.float32(BETA) * kl

    mask = completion_mask.astype(np.float32)
    mask_sum = max(mask.sum(), 1.0)
    kl_mean = (kl * mask).sum() / mask_sum
    is_clipped = (((coef_1 < 1.0 - EPS_LOW) & (adv < 0))
                  | ((coef_1 > 1.0 + EPS_HIGH) & (adv > 0)))
    clip_ratio = (is_clipped.astype(np.float32) * mask).sum() / mask_sum

    seq_lens = np.maximum(mask.sum(-1), 1.0)                     # (B,)
    reduced_loss = ((per_token_loss * mask).sum(-1) / seq_lens).mean()

    out = (np.float32(reduced_loss), np.float32(kl_mean), np.float32(clip_ratio))
    if _want_res:
        return out, res
    return out

